# revision 1
# baseline (speedup 1.0000x reference)
"""Trainium2 Bass kernel for nn_MetaSignatureEncoder (GCN encoder with FiLM
signature conditioning), distributed over 8 NeuronCores.

Strategy (graph/data parallel):
  - Nodes padded to NPAD = 50176, sharded contiguously (6272/core, 49 dst
    tiles of 128).  GCN norm factors: table rows are pre-scaled by dinv[src]
    on the host (xs = dinv*x, bf16); dinv[dst] is applied after aggregation.
    Self-loops are ordinary edges.
  - Pass 1 gathers xs rows (512B) straight from a replicated HBM table with
    dma_gather; NO phase-1 matmul and NO first AllGather.  The segment-sum
    runs on the TensorEngine in transposed orientation: for each 128-message
    chunk, matmul(lhsT=rows[:, f_half], rhs=S[msg, dst]) accumulates
    aggT[f, dst] in PSUM; S is a one-hot built by DVE is_equal vs iota.
    The fused weight [Wsig | W1] is applied AFTER aggregation (one matmul
    per tile instead of per-row work).
  - The node space is split in 3 segments (int16 gather indices); the table
    is stored segment-major so pass-2 AllGathers are per-segment slices that
    overlap the encoder.
  - Chunk counts per (tile, segment) are baked from the actual graph at
    compile time (max across cores); gather calls cover groups of G tiles
    with num_idxs_reg trimming of trailing padding.
  - Signature s is mask-matmul-reduced and AllReduced in f32; gamma/beta via
    f32 matmuls with s broadcast along the free axis.
  - Encoder: FiLM + relu + LN (bn_stats/bn_aggr) with dinv folded into the
    LN output scale -> h1' table, per-segment AllGather, pass 2 identical in
    structure (gather h1' rows, segsum, W2 after, FiLM + LN epilogue).

kernel(**inputs) takes the FULL problem inputs and returns the FULL output.
"""
import sys
import numpy as np
import ml_dtypes

sys.path.insert(0, "/opt/trn_rl_repo")

from concourse import bass, bacc, tile, mybir
from concourse import bass_utils

BF16 = ml_dtypes.bfloat16
dt = mybir.dt

# ---------------------------------------------------------------- config ----

NC = 8
TP = 128
NT = 49
SHARD = NT * TP          # 6272
NPAD = NC * SHARD        # 50176
IN_CH = 256
HID = 256
OUT = 128
FUSED = HID + HID        # sig(256) | conv1(256)
KA = 3                   # K chunks for augmented fc matmuls
LN_EPS = 1e-5
N_REAL = 50000

# segment tile-ranges (per-core tiles grouped into 3 segments)
T0 = [0, 17, 33, 49]
LENS = [17, 16, 16]
SEG_ROWS = [NC * L * TP for L in LENS]            # 17408, 16384, 16384
SEG_BASE = [0, SEG_ROWS[0], SEG_ROWS[0] + SEG_ROWS[1]]

G = 1                    # dst tiles per gather call
NQ = 4                   # SWDGE queues to round-robin
SCRATCH = 32768          # dynamic DMA scratch (ring carveout)
GAT_BUFS = 8
CH_CAP = 8               # max chunks (num_idxs capped at 1008)

# ------------------------------------------------------------ host side -----


def _wrap16(vals, nrows=128):
    n = vals.shape[0]
    assert n % 16 == 0
    w = vals.reshape(n // 16, 16).T
    return np.tile(w, (nrows // 16, 1))


def _pmaj(vals):
    return np.ascontiguousarray(vals.reshape(-1, TP).T)


def _seg_of_tile():
    s = np.zeros(NT, np.int64)
    s[T0[1]:T0[2]] = 1
    s[T0[2]:] = 2
    return s


def preprocess(edge_index):
    """Graph preprocessing -> shared static structure + per-core tables.

    Self-loops are NOT in the gather stream: their rows are each tile's own
    contiguous table rows, added on-device via identity matmuls.  They DO
    count toward deg (reference adds self-loops before computing the norm).
    """
    src = np.asarray(edge_index[0], dtype=np.int64)
    dst = np.asarray(edge_index[1], dtype=np.int64)

    deg = np.bincount(src, minlength=NPAD).astype(np.float32)
    deg[:N_REAL] += 1.0                            # self-loops
    # pad nodes have deg 0; floor at 1 so the device's 1/sqrt(deg) stays
    # finite (their agg is 0 and their outputs are cropped anyway)
    deg = np.where(deg > 0, deg, 1.0).astype(np.float32)
    dinv = np.where(deg > 0, deg ** -0.5, 0.0).astype(np.float32)

    seg_of = _seg_of_tile()
    # segment-major position of every node id
    n = np.arange(NPAD, dtype=np.int64)
    c_ = n // SHARD
    w_ = n % SHARD
    t_ = w_ // TP
    p_ = w_ % TP
    j_ = seg_of[t_]
    pos = c_ * (np.array(LENS)[j_] * TP) + (t_ - np.array(T0)[:3][j_]) * TP \
        + p_ + np.array(SEG_BASE)[j_]

    # per-core edge buckets by (dst tile, src segment), sorted by src pos
    counts = np.zeros((NC, NT, 3), np.int64)
    buckets = [[[None] * 3 for _ in range(NT)] for _ in range(NC)]
    shard_of = dst // SHARD
    for c in range(NC):
        m = shard_of == c
        s_c, d_c = src[m], dst[m] - c * SHARD
        dt_ = d_c // TP
        dp = d_c % TP
        sj = j_[s_c]
        sp = pos[s_c]
        order = np.lexsort((sp, sj, dt_))
        dt_, dp, sj, sp = dt_[order], dp[order], sj[order], sp[order]
        tb = np.searchsorted(dt_, np.arange(NT + 1))
        for t in range(NT):
            sl = slice(tb[t], tb[t + 1])
            sj_t, sp_t, dp_t = sj[sl], sp[sl], dp[sl]
            jb = np.searchsorted(sj_t, np.arange(4))
            for j in range(3):
                s2 = slice(jb[j], jb[j + 1])
                buckets[c][t][j] = (sp_t[s2], dp_t[s2])
                counts[c, t, j] = jb[j + 1] - jb[j]

    reg = counts.max(axis=0)                      # [NT, 3]
    reg16 = ((reg + 15) // 16) * 16
    # single_packet mode: per-call data packets round up to 128 and must
    # stay <= 63*16 per engine -> effective rows per call <= 896
    if reg16.max() > 896:
        raise OverflowError(f"count overflow {reg16.max()} > 896")
    assert reg16.min() > 0
    chunks = (reg16 + TP - 1) // TP               # [NT, 3]
    if chunks.max() > CH_CAP:
        raise OverflowError(f"chunk overflow {chunks.max()} > {CH_CAP}")

    # static call layout: one call per (tile, segment), num_idxs == reg16
    # (multiple of 16, not necessarily of 128; the last chunk is partial)
    assert G == 1
    calls = []
    idx_col = 0
    nchunks_total = 0
    for t in range(NT):
        for j in range(3):
            cap = int(reg16[t, j])
            calls.append(dict(
                j=j, col0=idx_col, ncols=cap // 16, nidx=cap,
                reg=cap, chunk0=nchunks_total,
                tiles=[(t, int(chunks[t, j]))]))
            idx_col += cap // 16
            nchunks_total += int(chunks[t, j])
    idx_cols_total = idx_col

    # per-core idx + seg tables
    per_core = []
    for c in range(NC):
        idx = np.zeros(idx_cols_total * 16, np.int64)
        seg = -np.ones((nchunks_total, TP), np.float32)
        for call in calls:
            j = call["j"]
            base = call["col0"] * 16
            ch = call["chunk0"]
            (t, nch), = call["tiles"]
            sp_t, dp_t = buckets[c][t][j]
            na = len(sp_t)
            assert na <= call["nidx"]
            idx[base: base + na] = sp_t - SEG_BASE[j]
            idx[base + na: base + call["nidx"]] = 0   # pad: gather row 0
            fl = seg[ch:ch + nch].reshape(-1)
            fl[:na] = dp_t
        per_core.append({
            "idx": _wrap16(idx).astype(np.int16),
            "seg": np.ascontiguousarray(seg.T).astype(BF16),
        })

    meta = dict(calls=calls, chunks=chunks, idx_cols=idx_cols_total,
                nchunks=nchunks_total)
    return deg, dinv, meta, per_core


_PRE = {}


def get_pre(edge_index):
    key = hash(np.asarray(edge_index)[:, ::1007].tobytes())
    if key not in _PRE:
        _PRE[key] = preprocess(edge_index)
    return _PRE[key]


def make_in_maps(inputs, meta, deg, dinv, per_core):
    x = np.asarray(inputs["x"], np.float32)
    xp = np.zeros((NPAD, IN_CH), np.float32)
    xp[: x.shape[0]] = x
    xs = xp * dinv[:, None]

    # segment-major reorder of the table
    seg_of = _seg_of_tile()
    n = np.arange(NPAD, dtype=np.int64)
    c_ = n // SHARD
    w_ = n % SHARD
    t_ = w_ // TP
    p_ = w_ % TP
    j_ = seg_of[t_]
    pos = SEG_BASE[0] * 0 + c_ * (np.array(LENS)[j_] * TP) \
        + (t_ - np.array(T0)[:3][j_]) * TP + p_ + np.array(SEG_BASE)[j_]
    xs_seg = np.zeros_like(xs)
    xs_seg[pos] = xs
    xs_seg = xs_seg.astype(BF16)

    def chunks_(a, k):
        return np.ascontiguousarray(a.reshape(k, 128, a.shape[1]))

    wf = np.concatenate([np.asarray(inputs["sig_conv_w"], np.float32),
                         np.asarray(inputs["conv1_w"], np.float32)], axis=1)

    def aug(w, b):
        wt = np.asarray(w, np.float32).T
        a = np.zeros((KA * 128, wt.shape[1]), np.float32)
        a[: wt.shape[0]] = wt
        a[wt.shape[0]] = np.asarray(b, np.float32)
        return chunks_(a, KA)

    gids = np.arange(SHARD)
    shared = {
        "xs": xs_seg,
        "ident": np.eye(128, dtype=np.float32).astype(BF16),
        "wf": chunks_(wf, 2).astype(BF16),
        "w2": chunks_(np.asarray(inputs["conv2_w"], np.float32), 2).astype(BF16),
        "wg1": aug(inputs["fc1_w"], inputs["fc1_b"]),
        "wb1": aug(inputs["fc2_w"], inputs["fc2_b"]),
        "wg2": aug(inputs["fc3_w"], inputs["fc3_b"]),
        "wb2": aug(inputs["fc4_w"], inputs["fc4_b"]),
        "bsig": np.broadcast_to(np.asarray(inputs["sig_conv_b"], np.float32),
                                (128, HID)).copy(),
        "b1c": np.broadcast_to(np.asarray(inputs["conv1_b"], np.float32),
                               (128, HID)).copy(),
        "b2c": np.broadcast_to(np.asarray(inputs["conv2_b"], np.float32),
                               (128, OUT)).copy(),
        "iota": np.broadcast_to(np.arange(128, dtype=np.float32),
                                (128, 128)).astype(BF16).copy(),
    }
    in_maps = []
    for c in range(NC):
        sl = slice(c * SHARD, (c + 1) * SHARD)
        m = dict(shared)
        m["deg"] = _pmaj(deg[sl]).copy()
        m["sigmask"] = _pmaj(((gids + c * SHARD) < N_REAL)
                             .astype(np.float32)).astype(BF16)
        m["xself"] = np.ascontiguousarray(xs[sl]).astype(BF16)
        m.update(per_core[c])
        in_maps.append(m)
    return in_maps

# --------------------------------------------------------------- builder ----


def build_program(meta):
    nc = bacc.Bacc("TRN2", target_bir_lowering=False, debug=False,
                   num_devices=NC, num_swdge_queues=NQ,
                   dynamic_dma_scratch_size=SCRATCH)
    f32, bf16, i16 = dt.float32, dt.bfloat16, dt.int16
    f8 = dt.float8e4
    calls = meta["calls"]
    IDXC = meta["idx_cols"]
    NCH = meta["nchunks"]
    CH_MAX = max(-(-c["nidx"] // TP) for c in calls)

    def inp(name, shape, dtype):
        return nc.dram_tensor(name, shape, dtype, kind="ExternalInput")

    xs_d = inp("xs", [NPAD, IN_CH], bf16)
    xself_d = inp("xself", [SHARD, IN_CH], bf16)
    ident_d = inp("ident", [TP, TP], bf16)
    wf_d = inp("wf", [2, TP, FUSED], bf16)
    w2_d = inp("w2", [2, TP, OUT], bf16)
    wg1_d = inp("wg1", [KA, TP, HID], f32)
    wb1_d = inp("wb1", [KA, TP, HID], f32)
    wg2_d = inp("wg2", [KA, TP, OUT], f32)
    wb2_d = inp("wb2", [KA, TP, OUT], f32)
    bsig_d = inp("bsig", [TP, HID], f32)
    b1c_d = inp("b1c", [TP, HID], f32)
    b2c_d = inp("b2c", [TP, OUT], f32)
    iota_d = inp("iota", [TP, TP], bf16)
    deg_d = inp("deg", [TP, NT], f32)
    mask_d = inp("sigmask", [TP, NT], bf16)
    idx_d = inp("idx", [TP, IDXC], i16)
    seg_d = inp("seg", [TP, NCH], bf16)

    out_d = nc.dram_tensor("out", [SHARD, OUT], f32, kind="ExternalOutput")

    tsh_d = nc.dram_tensor("tsh", [SHARD, HID], f8)
    tfull_sd = [nc.dram_tensor(f"tfull{j}", [SEG_ROWS[j], HID], f8,
                               addr_space="Shared") for j in range(3)]
    sin_d = nc.dram_tensor("sin", [1, HID], f32)
    sout_d = nc.dram_tensor("sout", [1, HID], f32, addr_space="Shared")

    rg = [list(range(NC))]

    with tile.TileContext(nc) as tc:
        with (
            tc.tile_pool(name="const", bufs=1) as const,
            tc.tile_pool(name="persist", bufs=1) as persist,
            tc.tile_pool(name="gat", bufs=GAT_BUFS) as gat,
            tc.tile_pool(name="sbuild", bufs=4) as sbuild,
            tc.tile_pool(name="epi", bufs=4) as epi,
            tc.tile_pool(name="small", bufs=8) as small,
            tc.tile_pool(name="one", bufs=1) as one,
            tc.tile_pool(name="ps_t", bufs=4, space="PSUM") as ps_t,
            tc.tile_pool(name="ps_pre", bufs=2, space="PSUM") as ps_pre,
            tc.tile_pool(name="ps_sig", bufs=1, space="PSUM") as ps_sig,
        ):
            # ---- constants (gather tables first: pass 1 needs them) -----
            idx_sb = const.tile([TP, IDXC], i16)
            seg_sb = const.tile([TP, NCH], bf16)
            iota_sb = const.tile([TP, TP], bf16)
            nc.sync.dma_start(out=idx_sb[:], in_=idx_d.ap())
            nc.sync.dma_start(out=seg_sb[:], in_=seg_d.ap())
            nc.sync.dma_start(out=iota_sb[:], in_=iota_d.ap())
            wf_sb = const.tile([TP, 2, FUSED], bf16)
            w2_sb = const.tile([TP, 2, OUT], bf16)
            nc.sync.dma_start(out=wf_sb[:], in_=wf_d.ap().transpose([1, 0, 2]))
            nc.sync.dma_start(out=w2_sb[:], in_=w2_d.ap().transpose([1, 0, 2]))
            fc_sb = {}
            for nm, d, width in (("wg1", wg1_d, HID), ("wb1", wb1_d, HID),
                                 ("wg2", wg2_d, OUT), ("wb2", wb2_d, OUT)):
                t_ = const.tile([TP, KA, width], f32, name=nm)
                nc.sync.dma_start(out=t_[:], in_=d.ap().transpose([1, 0, 2]))
                fc_sb[nm] = t_
            bsig_sb = const.tile([TP, HID], f32)
            b1c_sb = const.tile([TP, HID], f32)
            b2c_sb = const.tile([TP, OUT], f32)
            ident_sb = const.tile([TP, TP], bf16)
            deg_sb = const.tile([TP, NT], f32)
            mask_sb = const.tile([TP, NT], bf16)
            for t_, d in ((bsig_sb, bsig_d), (b1c_sb, b1c_d), (b2c_sb, b2c_d),
                          (ident_sb, ident_d),
                          (deg_sb, deg_d), (mask_sb, mask_d)):
                nc.sync.dma_start(out=t_[:], in_=d.ap())

            eps_sb = const.tile([TP, 1], f32)
            nc.vector.memset(eps_sb[:], LN_EPS)
            dinv_sb = const.tile([TP, NT], f32)
            nc.scalar.sqrt(dinv_sb[:], deg_sb[:])
            nc.vector.reciprocal(dinv_sb[:], dinv_sb[:])

            c1agg_sb = persist.tile([TP, NT, HID], bf16)
            h1self_sb = persist.tile([TP, NT, HID], f8)
            ident8_sb = persist.tile([TP, TP], f8)
            nc.scalar.activation(ident8_sb[:], ident_sb[:],
                                 mybir.ActivationFunctionType.Identity)

            # memset gather bufs once (stale NaN guard: S=0 * NaN = NaN)
            for b in range(GAT_BUFS):
                gz = gat.tile([TP, CH_MAX, IN_CH], bf16, tag="g256",
                              name=f"gz_{b}")
                nc.vector.memset(gz[:], 0.0)
            for b in range(GAT_BUFS):
                gz = gat.tile([TP, CH_MAX, HID], f8, tag="g256f8",
                              name=f"gz8_{b}")
                nc.vector.memset(gz[:], 0.0)

            qctr = [0]

            def self_src_pass1(t):
                xt = gat.tile([TP, IN_CH], bf16, tag="xself", name=f"xsf_{t}")
                nc.sync.dma_start(out=xt[:],
                                  in_=xself_d.ap()[t * TP:(t + 1) * TP, :])
                return xt[:]

            def self_src_pass2(t):
                return h1self_sb[:, t, :]

            def edge_pass(table_d, width, scope, self_src):
                """Yields (tiles, {tile: psum [TP, 2, TP]}) per tile group."""
                out_ps = {}
                with nc.named_scope(scope):
                    for g0 in range(0, NT, G):
                        tiles = [t for t in range(g0, min(g0 + G, NT))]
                        gcalls = calls[(g0 // G) * 3:(g0 // G) * 3 + 3]
                        bufs = []
                        for call in gcalls:
                            j = call["j"]
                            ncall = call["nidx"]
                            nch = -(-ncall // TP)
                            gb = gat.tile([TP, CH_MAX, width], bf16,
                                          tag="g256",
                                          name=f"g_{scope}_{g0}_{j}")
                            nc.gpsimd.dma_gather(
                                out_ap=gb[:, :nch, :],
                                in_ap=table_d.ap()[
                                    SEG_BASE[j]:SEG_BASE[j] + SEG_ROWS[j], :],
                                idxs_ap=idx_sb[:, call["col0"]:
                                               call["col0"] + call["ncols"]],
                                num_idxs=ncall,
                                num_idxs_reg=call["reg"],
                                elem_size=width,
                                queue_num=qctr[0] % NQ,
                            )
                            qctr[0] += 1
                            S = sbuild.tile([TP, CH_MAX, TP], bf16, tag="S",
                                            name=f"S_{scope}_{g0}_{j}")
                            seg_col = seg_sb[:, call["chunk0"]:
                                             call["chunk0"] + nch]
                            nc.vector.tensor_tensor(
                                S[:, :nch, :],
                                seg_col.unsqueeze(2).to_broadcast(
                                    (TP, nch, TP)),
                                iota_sb[:].unsqueeze(1).to_broadcast(
                                    (TP, nch, TP)),
                                mybir.AluOpType.is_equal)
                            bufs.append((call, gb, S))
                        # per-tile psum accumulation across the 3 segments;
                        # the self-loop rows (identity matmul) open it
                        for t in tiles:
                            # full 2KB bank per tile: start=True clears
                            # has_written for the WHOLE bank
                            out_ps[t] = ps_t.tile([TP, 4, TP], f32, tag="psT",
                                                  name=f"ps_{scope}_{t}")
                            sap = self_src(t)
                            # start=True ONLY on the first matmul into the
                            # bank: it clears has_written for the WHOLE bank,
                            # so the h=1 region then overwrites-on-first-write
                            for h in range(2):
                                nc.tensor.matmul(
                                    out_ps[t][:, h, :],
                                    sap[:, h * TP:(h + 1) * TP],
                                    ident_sb[:], start=(h == 0), stop=False)
                        first = {t: False for t in tiles}
                        for ci, (call, gb, S) in enumerate(bufs):
                            kk = 0
                            for (t, nch) in call["tiles"]:
                                last_call = ci == 2
                                for k in range(nch):
                                    for h in range(width // TP):
                                        nc.tensor.matmul(
                                            out_ps[t][:, h, :],
                                            gb[:, kk, h * TP:(h + 1) * TP],
                                            S[:, kk, :],
                                            start=first[t],
                                            stop=(last_call and k == nch - 1))
                                    first[t] = False
                                    kk += 1
                        yield tiles, out_ps

            # ---- pass 1 ---------------------------------------------------
            s_ps = ps_sig.tile([1, 2 * HID], f32)  # full bank
            sig_t = [0]
            for tiles, out_ps in edge_pass(xs_d, IN_CH, "pass1",
                                           self_src_pass1):
                for t in tiles:
                    dv = dinv_sb[:, t:t + 1]
                    aggT = epi.tile([TP, 2, TP], bf16, tag="aggT",
                                    name=f"aggT_{t}")
                    nc.scalar.copy(aggT[:], out_ps[t][:, :2, :])
                    pre = ps_pre.tile([TP, FUSED], f32, tag="pre",
                                      name=f"pre_{t}")
                    for h in range(2):
                        nc.tensor.matmul(pre[:], aggT[:, h, :],
                                         wf_sb[:, h, :],
                                         start=(h == 0), stop=(h == 1))
                    # sig half
                    sig_f = epi.tile([TP, HID], f32, tag="sigf",
                                     name=f"sigf_{t}")
                    nc.vector.scalar_tensor_tensor(
                        sig_f[:], pre[:, :HID], dv, bsig_sb[:],
                        mybir.AluOpType.mult, mybir.AluOpType.add)
                    sig_b = epi.tile([TP, HID], bf16, tag="sigb",
                                     name=f"sigb_{t}")
                    nc.scalar.activation(sig_b[:], sig_f[:],
                                         mybir.ActivationFunctionType.Relu)
                    nc.tensor.matmul(s_ps[:, :HID], mask_sb[:, t:t + 1],
                                     sig_b[:],
                                     start=(sig_t[0] == 0),
                                     stop=(sig_t[0] == NT - 1))
                    sig_t[0] += 1
                    # conv1 half -> c1agg (dinv applied)
                    nc.scalar.activation(c1agg_sb[:, t, :], pre[:, HID:],
                                         mybir.ActivationFunctionType.Copy,
                                         scale=dv)

            # ---- signature ------------------------------------------------
            with nc.named_scope("signature"):
                s_sb = one.tile([1, HID], f32)
                nc.scalar.copy(s_sb[:], s_ps[:, :HID])
                nc.sync.dma_start(out=sin_d.ap(), in_=s_sb[:])
                nc.gpsimd.collective_compute(
                    "AllReduce", mybir.AluOpType.add, replica_groups=rg,
                    ins=[sin_d.ap().opt()], outs=[sout_d.ap().opt()])
                s_col = one.tile([TP, KA], f32)
                nc.vector.memset(s_col[:], 0.0)
                nc.vector.memset(s_col[0:1, KA - 1:KA], 1.0)
                nc.sync.dma_start(
                    out=s_col[:, 0:2],
                    in_=sout_d.ap().rearrange("o (c p) -> (o c) p", p=TP)
                        .transpose([1, 0]))
                s_rep = one.tile([TP, KA, TP], f32)
                for c in range(KA):
                    nc.vector.tensor_copy(
                        s_rep[:, c, :],
                        s_col[:, c:c + 1].to_broadcast((TP, TP)))
                gb_sb = {}
                for nm, width in (("wg1", HID), ("wb1", HID),
                                  ("wg2", OUT), ("wb2", OUT)):
                    ps_fc = ps_pre.tile([TP, FUSED], f32, tag="pre", name=nm)
                    for c in range(KA):
                        nc.tensor.matmul(ps_fc[:, :width], s_rep[:, c, :],
                                         fc_sb[nm][:, c, :],
                                         start=(c == 0), stop=(c == KA - 1))
                    gb = one.tile([TP, width], f32, name=f"gb_{nm}", tag=nm)
                    nc.scalar.activation(gb[:], ps_fc[:, :width],
                                         mybir.ActivationFunctionType.Tanh)
                    gb_sb[nm] = gb
                nc.vector.tensor_tensor(gb_sb["wb1"][:], gb_sb["wb1"][:],
                                        b1c_sb[:], mybir.AluOpType.add)
                nc.vector.tensor_tensor(gb_sb["wb2"][:], gb_sb["wb2"][:],
                                        b2c_sb[:], mybir.AluOpType.add)

            # ---- encoder local + chunked AllGather ------------------------
            def ln_scale(src_ap, extra_scale):
                """bn stats -> (rstd*extra, -mu*rstd*extra) per partition."""
                st6 = small.tile([TP, 6], f32, tag="st6", name="st6")
                mv = small.tile([TP, 2], f32, tag="mv", name="mv")
                nc.vector.bn_stats(st6[:], src_ap)
                nc.vector.bn_aggr(mv[:], st6[:])
                std = small.tile([TP, 1], f32, tag="std", name="std")
                nc.scalar.activation(std[:], mv[:, 1:2],
                                     mybir.ActivationFunctionType.Sqrt,
                                     bias=eps_sb[:, 0:1])
                rstd = small.tile([TP, 1], f32, tag="rstd", name="rstd")
                nc.vector.reciprocal(rstd[:], std[:])
                if extra_scale is not None:
                    nc.vector.tensor_tensor(rstd[:], rstd[:], extra_scale,
                                            mybir.AluOpType.mult)
                nmr = small.tile([TP, 1], f32, tag="nmr", name="nmr")
                nc.vector.scalar_tensor_tensor(
                    nmr[:], mv[:, 0:1], -1.0, rstd[:],
                    mybir.AluOpType.mult, mybir.AluOpType.mult)
                return rstd, nmr

            with nc.named_scope("encoder_local"):
                for t in range(NT):
                    dv = dinv_sb[:, t:t + 1]
                    h_f = epi.tile([TP, HID], f32, tag="hf", name=f"h_{t}")
                    nc.vector.tensor_tensor(h_f[:], c1agg_sb[:, t, :],
                                            gb_sb["wg1"][:],
                                            mybir.AluOpType.mult)
                    nc.vector.tensor_tensor(h_f[:], h_f[:], gb_sb["wb1"][:],
                                            mybir.AluOpType.add)
                    nc.scalar.activation(h_f[:], h_f[:],
                                         mybir.ActivationFunctionType.Relu)
                    rstd, nmr = ln_scale(h_f[:], dv)
                    nc.scalar.activation(h1self_sb[:, t, :], h_f[:],
                                         mybir.ActivationFunctionType.Identity,
                                         bias=nmr[:, 0:1], scale=rstd[:, 0:1])
                    nc.sync.dma_start(out=tsh_d.ap()[t * TP:(t + 1) * TP, :],
                                      in_=h1self_sb[:, t, :])
                    for j in range(3):
                        if t == T0[j + 1] - 1:
                            nc.gpsimd.collective_compute(
                                "AllGather", mybir.AluOpType.bypass,
                                replica_groups=rg,
                                ins=[tsh_d.ap()[T0[j] * TP:T0[j + 1] * TP, :]
                                     .opt()],
                                outs=[tfull_sd[j].ap().opt()])

            # ---- pass 2: segment-major so gathers for segment j only wait
            # on AllGather j (the three AGs serialize on the collective
            # queue); per-(tile, segment) partial sums accumulate in SBUF,
            # reusing c1agg's buffer (fully consumed by the encoder)
            agg2_sb = c1agg_sb
            with nc.named_scope("pass2"):
                for j in range(3):
                    for t in range(NT):
                        call = calls[t * 3 + j]
                        ncall = call["nidx"]
                        nch = -(-ncall // TP)
                        gb = gat.tile([TP, CH_MAX, HID], f8, tag="g256f8",
                                      name=f"g2_{j}_{t}")
                        nc.gpsimd.dma_gather(
                            out_ap=gb[:, :nch, :],
                            in_ap=tfull_sd[j].ap(),
                            idxs_ap=idx_sb[:, call["col0"]:
                                           call["col0"] + call["ncols"]],
                            num_idxs=ncall,
                            num_idxs_reg=call["reg"],
                            elem_size=HID,
                            queue_num=qctr[0] % NQ,
                        )
                        qctr[0] += 1
                        S = sbuild.tile([TP, CH_MAX, TP], f8, tag="S8",
                                        name=f"S2_{j}_{t}")
                        seg_col = seg_sb[:, call["chunk0"]:
                                         call["chunk0"] + nch]
                        nc.vector.tensor_tensor(
                            S[:, :nch, :],
                            seg_col.unsqueeze(2).to_broadcast((TP, nch, TP)),
                            iota_sb[:].unsqueeze(1).to_broadcast(
                                (TP, nch, TP)),
                            mybir.AluOpType.is_equal)
                        psT = ps_t.tile([TP, 4, TP], f32, tag="psT",
                                        name=f"ps2_{j}_{t}")
                        started = False
                        if j == 0:
                            for h in range(2):
                                nc.tensor.matmul(
                                    psT[:, h, :],
                                    h1self_sb[:, t, h * TP:(h + 1) * TP],
                                    ident8_sb[:], start=(h == 0), stop=False)
                            started = True
                        for k in range(nch):
                            for h in range(2):
                                nc.tensor.matmul(
                                    psT[:, h, :],
                                    gb[:, k, h * TP:(h + 1) * TP],
                                    S[:, k, :],
                                    start=(not started and k == 0 and h == 0),
                                    stop=(k == nch - 1))
                        for h in range(2):
                            dst = agg2_sb[:, t, h * TP:(h + 1) * TP]
                            if j == 0:
                                nc.scalar.copy(dst, psT[:, h, :])
                            else:
                                nc.vector.tensor_tensor(
                                    dst, dst, psT[:, h, :],
                                    mybir.AluOpType.add)
                        if j == 2:
                            dv = dinv_sb[:, t:t + 1]
                            pre2 = ps_pre.tile([TP, FUSED], f32, tag="pre",
                                               name=f"pre2_{t}")
                            for h in range(2):
                                nc.tensor.matmul(
                                    pre2[:, :OUT],
                                    agg2_sb[:, t, h * TP:(h + 1) * TP],
                                    w2_sb[:, h, :],
                                    start=(h == 0), stop=(h == 1))
                            o_f = epi.tile([TP, OUT], f32, tag="of",
                                           name=f"o_{t}")
                            nc.vector.scalar_tensor_tensor(
                                o_f[:], pre2[:, :OUT], dv, gb_sb["wg2"][:],
                                mybir.AluOpType.mult, mybir.AluOpType.mult)
                            nc.vector.tensor_tensor(o_f[:], o_f[:],
                                                    gb_sb["wb2"][:],
                                                    mybir.AluOpType.add)
                            rstd, nmr = ln_scale(o_f[:], None)
                            o_ln = epi.tile([TP, OUT], f32, tag="oln",
                                            name=f"ol_{t}")
                            nc.scalar.activation(
                                o_ln[:], o_f[:],
                                mybir.ActivationFunctionType.Identity,
                                bias=nmr[:, 0:1], scale=rstd[:, 0:1])
                            nc.sync.dma_start(
                                out=out_d.ap()[t * TP:(t + 1) * TP, :],
                                in_=o_ln[:])

    nc.compile()
    return nc

# ---------------------------------------------------------------- runner ----


_CACHE = {}


def run(inputs, trace=False, **kw):
    deg, dinv, meta, per_core = get_pre(np.asarray(inputs["edge_index"]))
    key = ("v2", meta["idx_cols"], meta["nchunks"])
    if key not in _CACHE:
        _CACHE[key] = build_program(meta)
    nc = _CACHE[key]
    in_maps = make_in_maps(inputs, meta, deg, dinv, per_core)
    res = bass_utils.run_bass_kernel_spmd(
        nc, in_maps, core_ids=list(range(NC)), trace=trace, **kw)
    out = np.concatenate([res.results[c]["out"] for c in range(NC)],
                         axis=0)[:N_REAL]
    return out.astype(np.float32), res


def kernel(**inputs):
    out, _ = run(inputs)
    return out


FULL = None  # compat with test.py signature

